# revision 23
# baseline (speedup 1.0000x reference)
"""Multi-head attention (B=2, S=2048, D=1024, H=16) on 8 TRN2 NeuronCores.

Tensor-parallel over heads: core c owns heads {2c, 2c+1} (a 128-wide slice of
the qkv projections / a 128-row slice of Wo). Each core computes its partial
out-projection; the host sums the 8 partials and adds the bias.

Per-core pipeline (all matmuls in float32r, ~1.6e-4 rel err, full PE rate):
  - QT/KT = (q @ Wq|k)^T in [c, s] layout (lhsT = W chunk, rhs = qT chunk)
  - VT likewise, then PE-transposed into vh [k, dv] tiles augmented with a
    ones column so the attn@V matmul also produces the softmax row-sums
  - scores computed transposed: sT[k, q] = KT_h.T-slice @ QT_h (contraction
    over head dim, both heads packed into the PE array via row tiling)
  - exp on ScalarE with fused 1/8 scale, no max subtraction (scores ~N(0,1))
  - AV: ctxT_u[dv+1, q] accumulated over 16 k-tiles
  - normalization: reciprocal of the rowsum row, K=1 ones-matmul broadcast
    across partitions, DVE multiply; head-1 lanes moved to partitions 64:128
    with 32-aligned cross-quadrant DVE copies
  - out projection: out[s, e] partial = ctxT_n.T-slice @ Wo_slice
"""

import numpy as np

import concourse.bass as bass
import concourse.mybir as mybir
import concourse.tile as tile
from concourse import bacc
from concourse.bass_utils import run_bass_kernel_spmd
from concourse.masks import make_identity

F32 = mybir.dt.float32
F32R = mybir.dt.float32r
EXP = mybir.ActivationFunctionType.Exp

B, S, D = 2, 2048, 1024
H, DH = 16, 64
NCORES = 8
C = (H // NCORES) * DH  # per-core ctx width = 128
BS = B * S  # 4096

_CACHED_NC = None


def _build():
    nc = bacc.Bacc("TRN2", target_bir_lowering=False, debug=False)

    qT_d = nc.dram_tensor("qT", [D, BS], F32R, kind="ExternalInput")
    # weights come host-prearranged as [128, 8*C]: partition-major chunks so
    # each SBUF partition loads one contiguous 4KB line
    wq_d = nc.dram_tensor("wq", [128, 8 * C], F32R, kind="ExternalInput")
    wk_d = nc.dram_tensor("wk", [128, 8 * C], F32R, kind="ExternalInput")
    wv_d = nc.dram_tensor("wv", [128, 8 * C], F32R, kind="ExternalInput")
    wo_d = nc.dram_tensor("wo", [C, D], F32R, kind="ExternalInput")
    out_d = nc.dram_tensor("out", [BS, D], F32, kind="ExternalOutput")

    with tile.TileContext(nc) as tc:
        with (
            tc.tile_pool(name="cp", bufs=1) as cp,
            tc.tile_pool(name="pp", bufs=1) as pp,
            tc.tile_pool(name="sp", bufs=4) as sp,
            tc.tile_pool(name="wp", bufs=2) as wp,
            tc.tile_pool(name="ps", bufs=2, space="PSUM") as ps,
        ):
            ident = cp.tile([128, 128], F32, tag="ident")
            make_identity(nc, ident[:])
            ones32 = cp.tile([65, 64], F32, tag="ones32")
            nc.vector.memset(ones32[:], 1.0)
            ones_bc = cp.tile([65, 64], F32R, tag="ones_bc")
            nc.vector.tensor_copy(ones_bc[:], ones32[:])
            onesv = cp.tile([128, 16], F32, tag="onesv")
            nc.vector.memset(onesv[:], 1.0)

            def load_qt(b, sbi, qt_t=None, chunks=range(8)):
                s0 = sbi * 512
                if qt_t is None:
                    qt_t = sp.tile(
                        [128, 8, 512], F32R, tag="qt", bufs=2, name=f"qt_{b}_{sbi}"
                    )
                for kc in chunks:
                    nc.sync.dma_start(
                        qt_t[:, kc, :],
                        qT_d[
                            kc * 128 : (kc + 1) * 128, b * S + s0 : b * S + s0 + 512
                        ],
                    )
                return qt_t

            # DMA order at start is critical (queues drain roughly in issue
            # order): first qT chunk, then wq so the first projection matmul
            # can start ~2us in, then the rest.
            qt_first = load_qt(0, 0, chunks=range(1))
            wq_sb = cp.tile([128, 8, C], F32R, tag="wq")
            wk_sb = cp.tile([128, 8, C], F32R, tag="wk")
            wv_sb = cp.tile([128, 8, C], F32R, tag="wv")
            nc.sync.dma_start(wq_sb[:, 0, :], wq_d[:, 0:C])
            nc.sync.dma_start(
                wq_sb[:, 1:8, :],
                wq_d[:, C : 8 * C].rearrange("p (o c) -> p o c", o=7),
            )
            load_qt(0, 0, qt_t=qt_first, chunks=range(1, 8))
            nc.sync.dma_start(wk_sb[:], wk_d.ap().rearrange("p (o c) -> p o c", o=8))
            nc.sync.dma_start(wv_sb[:], wv_d.ap().rearrange("p (o c) -> p o c", o=8))
            wo_sb = cp.tile([128, D], F32R, tag="wo")
            nc.sync.dma_start(wo_sb[:], wo_d[:, :])

            QT = [pp.tile([128, S], F32R, tag=f"QT{b}", name=f"QT{b}") for b in range(B)]
            KT = [pp.tile([128, S], F32R, tag=f"KT{b}", name=f"KT{b}") for b in range(B)]
            vh = [
                pp.tile([128, 16, 130], F32R, tag=f"vh{b}", name=f"vh{b}")
                for b in range(B)
            ]
            strip = [
                [
                    pp.tile(
                        [128, 16, 256], F32R, tag=f"strip{h}{p}", name=f"strip{h}{p}"
                    )
                    for p in range(2)
                ]
                for h in range(2)
            ]

            def proj(b, sbi, qt_t=None):
                """Project one 512-wide s-block of batch b into QT/KT and vh."""
                s0 = sbi * 512
                if qt_t is None:
                    qt_t = load_qt(b, sbi)
                for w_sb, dst in ((wq_sb, QT[b]), (wk_sb, KT[b])):
                    pt = ps.tile([128, 512], F32, tag="bcop", name=f"pj_{b}_{sbi}")
                    for kc in range(8):
                        nc.tensor.matmul(
                            pt[:],
                            w_sb[:, kc, :],
                            qt_t[:, kc, :],
                            start=(kc == 0),
                            stop=(kc == 7),
                        )
                    nc.vector.tensor_copy(dst[:, s0 : s0 + 512], pt[:])
                # V: project, then PE-transpose 128x128 tiles into vh
                pt = ps.tile([128, 512], F32, tag="bcop", name=f"pjv_{b}_{sbi}")
                for kc in range(8):
                    nc.tensor.matmul(
                        pt[:],
                        wv_sb[:, kc, :],
                        qt_t[:, kc, :],
                        start=(kc == 0),
                        stop=(kc == 7),
                    )
                vt_blk = sp.tile([128, 512], F32, tag="vt", bufs=2, name=f"vt_{b}_{sbi}")
                nc.vector.tensor_copy(vt_blk[:], pt[:])
                for t in range(4):
                    st = sbi * 4 + t
                    ptr = ps.tile([128, 128], F32, tag="bcop", name=f"vtr_{b}_{st}")
                    nc.tensor.transpose(
                        ptr[:], vt_blk[:, t * 128 : (t + 1) * 128], ident[:]
                    )
                    nc.vector.tensor_copy(
                        vh[b][:, st, 0:130].rearrange("p (g j) -> p g j", g=2, j=65)[
                            :, :, 0:64
                        ],
                        ptr[:].rearrange("p (g j) -> p g j", g=2, j=64),
                    )

            def vh_ones(b):
                nc.vector.tensor_copy(vh[b][:, :, 64], onesv[:])
                nc.vector.tensor_copy(vh[b][:, :, 129], onesv[:])

            QW = 256  # q-chunk width of the scores->exp->AV pipeline

            def scores_exp(b, c, par):
                """Scores + exp for q-chunk c into strip[h][par]."""
                q0 = c * QW
                for g in range(4):
                    for h in range(2):
                        hp = h * 64
                        pscr = ps.tile(
                            [128, 1024], F32, tag="scores", name=f"sc_{b}_{c}_{h}_{g}"
                        )
                        for j in range(4):
                            kt = g * 4 + j
                            nc.tensor.matmul(
                                pscr[:, j * QW : (j + 1) * QW],
                                KT[b][hp : hp + 64, kt * 128 : (kt + 1) * 128],
                                QT[b][hp : hp + 64, q0 : q0 + QW],
                                start=True,
                                stop=True,
                            )
                        nc.scalar.activation(
                            strip[h][par][:, 4 * g : 4 * g + 4, :],
                            pscr[:].rearrange("p (g j) -> p g j", g=4, j=QW),
                            EXP,
                            scale=0.125,
                        )

            def av_out(b, c, par):
                """attn@V (+rowsum), normalize, partial out-proj for chunk c."""
                pctx, pbc = [], []
                for h in range(2):
                    pc = ps.tile([65, QW], F32, tag="ctx", name=f"cx_{b}_{c}_{h}")
                    for kt in range(16):
                        nc.tensor.matmul(
                            pc[:],
                            vh[b][:, kt, h * 65 : (h + 1) * 65],
                            strip[h][par][:, kt, :],
                            start=(kt == 0),
                            stop=(kt == 15),
                        )
                    rc = wp.tile([65, QW], F32R, tag="rcp", name=f"rc_{b}_{c}_{h}")
                    with nc.allow_low_precision(reason="softmax denominator f32r"):
                        nc.vector.reciprocal(rc[64:65, :], pc[64:65, :])
                    pb = ps.tile([64, QW], F32, tag="bcop", name=f"bc_{b}_{c}_{h}")
                    nc.tensor.matmul(
                        pb[:], ones_bc[64:65, :], rc[64:65, :], start=True, stop=True
                    )
                    pctx.append(pc)
                    pbc.append(pb)

                ctxn = wp.tile([128, QW], F32R, tag="ctxn", name=f"cn_{b}_{c}")
                cu0 = wp.tile([64, QW], F32, tag="cu", name=f"cu0_{b}_{c}")
                nc.vector.tensor_copy(cu0[:], pctx[0][0:64, :])
                nc.vector.tensor_mul(ctxn[0:64, :], cu0[:], pbc[0][0:64, :])
                cu1 = wp.tile([64, QW], F32, tag="cu", name=f"cu1_{b}_{c}")
                nc.vector.tensor_copy(cu1[:], pctx[1][0:64, :])
                tm1 = wp.tile([64, QW], F32R, tag="tm1", bufs=1, name=f"tm1_{b}_{c}")
                nc.vector.tensor_mul(tm1[:], cu1[:], pbc[1][0:64, :])
                nc.vector.tensor_copy(ctxn[64:96, :], tm1[0:32, :])
                nc.vector.tensor_copy(ctxn[96:128, :], tm1[32:64, :])

                for sc in range(QW // 128):
                    ob = wp.tile([128, D], F32, tag="ob", name=f"ob_{b}_{c}_{sc}")
                    for eh in range(2):
                        po = ps.tile(
                            [128, 512], F32, tag="bcop", name=f"po_{b}_{c}_{sc}_{eh}"
                        )
                        nc.tensor.matmul(
                            po[:],
                            ctxn[:, sc * 128 : (sc + 1) * 128],
                            wo_sb[:, eh * 512 : (eh + 1) * 512],
                            start=True,
                            stop=True,
                        )
                        nc.vector.tensor_copy(ob[:, eh * 512 : (eh + 1) * 512], po[:])
                    r0 = b * S + c * QW + sc * 128
                    nc.sync.dma_start(out_d[r0 : r0 + 128, :], ob[:])

            vh_ones(0)
            vh_ones(1)
            proj(0, 0, qt_first)
            for sbi in range(1, 4):
                proj(0, sbi)
            NCH = S // QW  # chunks per batch
            for gc in range(2 * NCH):
                b, c = divmod(gc, NCH)
                scores_exp(b, c, gc % 2)
                if gc > 0:
                    pb_, pc_ = divmod(gc - 1, NCH)
                    av_out(pb_, pc_, (gc - 1) % 2)
                if gc % 2 == 1 and gc < NCH:
                    proj(1, gc // 2)
            av_out(1, NCH - 1, (2 * NCH - 1) % 2)

    nc.compile()
    return nc


def _get_nc():
    global _CACHED_NC
    if _CACHED_NC is None:
        _CACHED_NC = _build()
    return _CACHED_NC


def _in_maps(q, Wq, Wk, Wv, Wo):
    qT = np.ascontiguousarray(np.asarray(q, np.float32).reshape(BS, D).T)
    Wq = np.asarray(Wq, np.float32)
    Wk = np.asarray(Wk, np.float32)
    Wv = np.asarray(Wv, np.float32)
    Wo = np.asarray(Wo, np.float32)
    def warr(W, sl):
        # [D, C] slice -> [128, 8*C]: partition p holds chunks (o*128+p, :)
        w = W[:, sl].reshape(8, 128, C).transpose(1, 0, 2)
        return np.ascontiguousarray(w.reshape(128, 8 * C))

    maps = []
    for c in range(NCORES):
        sl = slice(c * C, (c + 1) * C)
        maps.append(
            {
                "qT": qT,
                "wq": warr(Wq, sl),
                "wk": warr(Wk, sl),
                "wv": warr(Wv, sl),
                "wo": np.ascontiguousarray(Wo[sl, :]),
            }
        )
    return maps


def run(q, Wq, Wk, Wv, Wo, bo, trace=False):
    nc = _get_nc()
    res = run_bass_kernel_spmd(
        nc, _in_maps(q, Wq, Wk, Wv, Wo), list(range(NCORES)), trace=trace
    )
    acc = np.zeros((BS, D), np.float64)
    for r in res.results:
        acc += r["out"]
    out = (acc + np.asarray(bo, np.float32).astype(np.float64)).astype(np.float32)
    return out.reshape(B, S, D), res


def kernel(q, Wq, Wk, Wv, Wo, bo):
    out, _ = run(q, Wq, Wk, Wv, Wo, bo)
    return out


# revision 24
# speedup vs baseline: 1.0147x; 1.0147x over previous
"""Multi-head attention (B=2, S=2048, D=1024, H=16) on 8 TRN2 NeuronCores.

Tensor-parallel over heads: core c owns heads {2c, 2c+1} (a 128-wide slice of
the qkv projections / a 128-row slice of Wo). Each core computes its partial
out-projection; the host sums the 8 partials and adds the bias.

Per-core pipeline (all matmuls in float32r, ~1.6e-4 rel err, full PE rate):
  - QT/KT = (q @ Wq|k)^T in [c, s] layout (lhsT = W chunk, rhs = qT chunk)
  - VT likewise, then PE-transposed into vh [k, dv] tiles augmented with a
    ones column so the attn@V matmul also produces the softmax row-sums
  - scores computed transposed: sT[k, q] = KT_h.T-slice @ QT_h (contraction
    over head dim, both heads packed into the PE array via row tiling)
  - exp on ScalarE with fused 1/8 scale, no max subtraction (scores ~N(0,1))
  - AV: ctxT_u[dv+1, q] accumulated over 16 k-tiles
  - normalization: reciprocal of the rowsum row, K=1 ones-matmul broadcast
    across partitions, DVE multiply; head-1 lanes moved to partitions 64:128
    with 32-aligned cross-quadrant DVE copies
  - out projection: out[s, e] partial = ctxT_n.T-slice @ Wo_slice
"""

import numpy as np

import concourse.bass as bass
import concourse.mybir as mybir
import concourse.tile as tile
from concourse import bacc
from concourse.bass_utils import run_bass_kernel_spmd
from concourse.masks import make_identity

F32 = mybir.dt.float32
F32R = mybir.dt.float32r
EXP = mybir.ActivationFunctionType.Exp

B, S, D = 2, 2048, 1024
H, DH = 16, 64
NCORES = 8
C = (H // NCORES) * DH  # per-core ctx width = 128
BS = B * S  # 4096

_CACHED_NC = None


def _build():
    nc = bacc.Bacc("TRN2", target_bir_lowering=False, debug=False)

    qT_d = nc.dram_tensor("qT", [D, BS], F32R, kind="ExternalInput")
    # weights come host-prearranged as [128, 8*C]: partition-major chunks so
    # each SBUF partition loads one contiguous 4KB line
    wq_d = nc.dram_tensor("wq", [128, 8 * C], F32R, kind="ExternalInput")
    wk_d = nc.dram_tensor("wk", [128, 8 * C], F32R, kind="ExternalInput")
    wv_d = nc.dram_tensor("wv", [128, 8 * C], F32R, kind="ExternalInput")
    wo_d = nc.dram_tensor("wo", [C, D], F32R, kind="ExternalInput")
    out_d = nc.dram_tensor("out", [BS, D], F32, kind="ExternalOutput")

    with tile.TileContext(nc) as tc:
        with (
            tc.tile_pool(name="cp", bufs=1) as cp,
            tc.tile_pool(name="pp", bufs=1) as pp,
            tc.tile_pool(name="sp", bufs=4) as sp,
            tc.tile_pool(name="wp", bufs=2) as wp,
            tc.tile_pool(name="ps", bufs=2, space="PSUM") as ps,
        ):
            ident = cp.tile([128, 128], F32, tag="ident")
            make_identity(nc, ident[:])
            ones32 = cp.tile([65, 64], F32, tag="ones32")
            nc.vector.memset(ones32[:], 1.0)
            ones_bc = cp.tile([65, 64], F32R, tag="ones_bc")
            nc.vector.tensor_copy(ones_bc[:], ones32[:])
            onesv = cp.tile([128, 16], F32, tag="onesv")
            nc.vector.memset(onesv[:], 1.0)

            def load_qt(b, sbi, qt_t=None, chunks=range(8)):
                s0 = sbi * 512
                if qt_t is None:
                    qt_t = sp.tile(
                        [128, 8, 512], F32R, tag="qt", bufs=2, name=f"qt_{b}_{sbi}"
                    )
                for kc in chunks:
                    nc.sync.dma_start(
                        qt_t[:, kc, :],
                        qT_d[
                            kc * 128 : (kc + 1) * 128, b * S + s0 : b * S + s0 + 512
                        ],
                    )
                return qt_t

            # DMA order at start is critical (queues drain roughly in issue
            # order): first qT chunk, then wq so the first projection matmul
            # can start ~2us in, then the rest.
            qt_first = load_qt(0, 0, chunks=range(1))
            wq_sb = cp.tile([128, 8, C], F32R, tag="wq")
            wk_sb = cp.tile([128, 8, C], F32R, tag="wk")
            wv_sb = cp.tile([128, 8, C], F32R, tag="wv")
            nc.sync.dma_start(wq_sb[:, 0, :], wq_d[:, 0:C])
            nc.sync.dma_start(
                wq_sb[:, 1:8, :],
                wq_d[:, C : 8 * C].rearrange("p (o c) -> p o c", o=7),
            )
            load_qt(0, 0, qt_t=qt_first, chunks=range(1, 8))
            nc.sync.dma_start(wk_sb[:], wk_d.ap().rearrange("p (o c) -> p o c", o=8))
            nc.sync.dma_start(wv_sb[:], wv_d.ap().rearrange("p (o c) -> p o c", o=8))
            wo_sb = cp.tile([128, D], F32R, tag="wo")
            nc.sync.dma_start(wo_sb[:], wo_d[:, :])

            QT = [pp.tile([128, S], F32R, tag=f"QT{b}", name=f"QT{b}") for b in range(B)]
            KT = [pp.tile([128, S], F32R, tag=f"KT{b}", name=f"KT{b}") for b in range(B)]
            vh = [
                pp.tile([128, 16, 130], F32R, tag=f"vh{b}", name=f"vh{b}")
                for b in range(B)
            ]
            strip = [
                [
                    pp.tile(
                        [128, 16, 256], F32R, tag=f"strip{h}{p}", name=f"strip{h}{p}"
                    )
                    for p in range(2)
                ]
                for h in range(2)
            ]

            def proj(b, sbi, qt_t=None):
                """Project one 512-wide s-block of batch b into QT/KT and vh."""
                s0 = sbi * 512
                if qt_t is None:
                    qt_t = load_qt(b, sbi)
                for w_sb, dst in ((wq_sb, QT[b]), (wk_sb, KT[b])):
                    pt = ps.tile([128, 512], F32, tag="bcop", name=f"pj_{b}_{sbi}")
                    for kc in range(8):
                        nc.tensor.matmul(
                            pt[:],
                            w_sb[:, kc, :],
                            qt_t[:, kc, :],
                            start=(kc == 0),
                            stop=(kc == 7),
                        )
                    nc.vector.tensor_copy(dst[:, s0 : s0 + 512], pt[:])
                # V: project, then PE-transpose 128x128 tiles into vh
                pt = ps.tile([128, 512], F32, tag="bcop", name=f"pjv_{b}_{sbi}")
                for kc in range(8):
                    nc.tensor.matmul(
                        pt[:],
                        wv_sb[:, kc, :],
                        qt_t[:, kc, :],
                        start=(kc == 0),
                        stop=(kc == 7),
                    )
                vt_blk = sp.tile([128, 512], F32, tag="vt", bufs=2, name=f"vt_{b}_{sbi}")
                nc.vector.tensor_copy(vt_blk[:], pt[:])
                for t in range(4):
                    st = sbi * 4 + t
                    ptr = ps.tile([128, 128], F32, tag="bcop", name=f"vtr_{b}_{st}")
                    nc.tensor.transpose(
                        ptr[:], vt_blk[:, t * 128 : (t + 1) * 128], ident[:]
                    )
                    nc.vector.tensor_copy(
                        vh[b][:, st, 0:130].rearrange("p (g j) -> p g j", g=2, j=65)[
                            :, :, 0:64
                        ],
                        ptr[:].rearrange("p (g j) -> p g j", g=2, j=64),
                    )

            def vh_ones(b):
                nc.vector.tensor_copy(vh[b][:, :, 64], onesv[:])
                nc.vector.tensor_copy(vh[b][:, :, 129], onesv[:])

            QW = 256  # q-chunk width of the scores->exp->AV pipeline

            def scores_exp(b, c, par):
                """Scores + exp for q-chunk c into strip[h][par]."""
                q0 = c * QW
                for g in range(4):
                    for h in range(2):
                        hp = h * 64
                        pscr = ps.tile(
                            [128, 1024], F32, tag="scores", name=f"sc_{b}_{c}_{h}_{g}"
                        )
                        for j in range(4):
                            kt = g * 4 + j
                            nc.tensor.matmul(
                                pscr[:, j * QW : (j + 1) * QW],
                                KT[b][hp : hp + 64, kt * 128 : (kt + 1) * 128],
                                QT[b][hp : hp + 64, q0 : q0 + QW],
                                start=True,
                                stop=True,
                            )
                        nc.scalar.activation(
                            strip[h][par][:, 4 * g : 4 * g + 4, :],
                            pscr[:].rearrange("p (g j) -> p g j", g=4, j=QW),
                            EXP,
                            scale=0.125,
                        )

            def av_out(b, c, par):
                """attn@V (+rowsum), normalize, partial out-proj for chunk c."""
                pctx = []
                rc = wp.tile([65, 2 * QW], F32R, tag="rcp", name=f"rc_{b}_{c}")
                for h in range(2):
                    pc = ps.tile([65, QW], F32, tag="ctx", name=f"cx_{b}_{c}_{h}")
                    for kt in range(16):
                        nc.tensor.matmul(
                            pc[:],
                            vh[b][:, kt, h * 65 : (h + 1) * 65],
                            strip[h][par][:, kt, :],
                            start=(kt == 0),
                            stop=(kt == 15),
                        )
                    with nc.allow_low_precision(reason="softmax denominator f32r"):
                        nc.vector.reciprocal(
                            rc[64:65, h * QW : (h + 1) * QW], pc[64:65, :]
                        )
                    pctx.append(pc)
                # one broadcast matmul covers both heads' reciprocal rows
                pball = ps.tile([64, 2 * QW], F32, tag="bcop", name=f"bc_{b}_{c}")
                nc.tensor.matmul(
                    pball[:], ones_bc[64:65, :], rc[64:65, :], start=True, stop=True
                )
                pbc = [pball[:, 0:QW], pball[:, QW : 2 * QW]]

                ctxn = wp.tile([128, QW], F32R, tag="ctxn", name=f"cn_{b}_{c}")
                cu0 = wp.tile([64, QW], F32, tag="cu", name=f"cu0_{b}_{c}")
                nc.vector.tensor_copy(cu0[:], pctx[0][0:64, :])
                nc.vector.tensor_mul(ctxn[0:64, :], cu0[:], pbc[0])
                cu1 = wp.tile([64, QW], F32, tag="cu", name=f"cu1_{b}_{c}")
                nc.vector.tensor_copy(cu1[:], pctx[1][0:64, :])
                tm1 = wp.tile([64, QW], F32R, tag="tm1", bufs=1, name=f"tm1_{b}_{c}")
                nc.vector.tensor_mul(tm1[:], cu1[:], pbc[1])
                nc.vector.tensor_copy(ctxn[64:96, :], tm1[0:32, :])
                nc.vector.tensor_copy(ctxn[96:128, :], tm1[32:64, :])

                for sc in range(QW // 128):
                    ob = wp.tile([128, D], F32, tag="ob", name=f"ob_{b}_{c}_{sc}")
                    for eh in range(2):
                        po = ps.tile(
                            [128, 512], F32, tag="bcop", name=f"po_{b}_{c}_{sc}_{eh}"
                        )
                        nc.tensor.matmul(
                            po[:],
                            ctxn[:, sc * 128 : (sc + 1) * 128],
                            wo_sb[:, eh * 512 : (eh + 1) * 512],
                            start=True,
                            stop=True,
                        )
                        nc.vector.tensor_copy(ob[:, eh * 512 : (eh + 1) * 512], po[:])
                    r0 = b * S + c * QW + sc * 128
                    nc.sync.dma_start(out_d[r0 : r0 + 128, :], ob[:])

            vh_ones(0)
            vh_ones(1)
            proj(0, 0, qt_first)
            for sbi in range(1, 4):
                proj(0, sbi)
            NCH = S // QW  # chunks per batch
            for gc in range(2 * NCH):
                b, c = divmod(gc, NCH)
                scores_exp(b, c, gc % 2)
                if gc > 0:
                    pb_, pc_ = divmod(gc - 1, NCH)
                    av_out(pb_, pc_, (gc - 1) % 2)
                if gc % 2 == 1 and gc < NCH:
                    proj(1, gc // 2)
            av_out(1, NCH - 1, (2 * NCH - 1) % 2)

    nc.compile()
    return nc


def _get_nc():
    global _CACHED_NC
    if _CACHED_NC is None:
        _CACHED_NC = _build()
    return _CACHED_NC


def _in_maps(q, Wq, Wk, Wv, Wo):
    qT = np.ascontiguousarray(np.asarray(q, np.float32).reshape(BS, D).T)
    Wq = np.asarray(Wq, np.float32)
    Wk = np.asarray(Wk, np.float32)
    Wv = np.asarray(Wv, np.float32)
    Wo = np.asarray(Wo, np.float32)
    def warr(W, sl):
        # [D, C] slice -> [128, 8*C]: partition p holds chunks (o*128+p, :)
        w = W[:, sl].reshape(8, 128, C).transpose(1, 0, 2)
        return np.ascontiguousarray(w.reshape(128, 8 * C))

    maps = []
    for c in range(NCORES):
        sl = slice(c * C, (c + 1) * C)
        maps.append(
            {
                "qT": qT,
                "wq": warr(Wq, sl),
                "wk": warr(Wk, sl),
                "wv": warr(Wv, sl),
                "wo": np.ascontiguousarray(Wo[sl, :]),
            }
        )
    return maps


def run(q, Wq, Wk, Wv, Wo, bo, trace=False):
    nc = _get_nc()
    res = run_bass_kernel_spmd(
        nc, _in_maps(q, Wq, Wk, Wv, Wo), list(range(NCORES)), trace=trace
    )
    acc = np.zeros((BS, D), np.float64)
    for r in res.results:
        acc += r["out"]
    out = (acc + np.asarray(bo, np.float32).astype(np.float64)).astype(np.float32)
    return out.reshape(B, S, D), res


def kernel(q, Wq, Wk, Wv, Wo, bo):
    out, _ = run(q, Wq, Wk, Wv, Wo, bo)
    return out
